# revision 8
# baseline (speedup 1.0000x reference)
"""Trainium2 Bass kernel for nn_AttBlock (LSTM cell + 3-head attention recurrence).

Hybrid sharding on 8 cores, fp16 matmul operands (fp32 accumulate):
  - attention + matvec: batch-sharded (core k owns batches 32k..32k+31)
  - gates matmul + LSTM state: H-sharded (core k owns H dims 128k..128k+127,
    i.e. gate rows {g*1024 + 128k + r})
  - two AllGathers per step: v^T (fp16) and h^T (fp16)

Per-core layouts (P = 128 partitions; b' = own batch, B = global batch):
  hT_full [128, 8, 256] fp16  h^T: partition p, col (r, B): h[B, 128r+p]
  c_st    [128, 256] fp32     c[B, 128k+p] (own H slice, all batches)
  x_sb    [128, 32*2*6*128] fp16  lhsT tiles col ((b'*2+kt)*6+dt)*128+d'
  wat_sb  [128, 8*768] fp16   W_att^T tiles: col kt*768 + mt*128 + m'
  wgs_sb  [128, 26*4*128] fp16 W_cat^T shard: col (kt*4+g)*128 + r
  vT_full [128, 8, 18, 32] fp16 v^T: partition p, col (r, q, b'): q = dt*3+i,
                              v-dim = i*768 + dt*128 + p, batch = 32r+b'
  psum_l  [128, 6*32] logits^T own batches: col (mt, b'), mt = i*2+lt
"""

import sys

sys.path.insert(0, "/opt/trn_rl_repo")

from contextlib import ExitStack

import numpy as np

import concourse.bass as bass
import concourse.tile as tile
from concourse import bacc
from concourse import mybir

B, L, D, H = 256, 256, 768, 1024
NCORES = 8
BL = B // NCORES  # 32
F32 = mybir.dt.float32
F16 = mybir.dt.float16
BF16 = mybir.dt.bfloat16
AF = mybir.ActivationFunctionType


def build_program(t_steps: int):
    nc = bacc.Bacc()
    xq = nc.declare_dram_parameter("xq", [128, BL * 2 * 6 * 128], F16, isOutput=False)
    wat = nc.declare_dram_parameter("wat", [128, 8 * 768], F16, isOutput=False)
    wgs = nc.declare_dram_parameter("wgs", [128, 26 * 4 * 128], F16, isOutput=False)
    boff = nc.declare_dram_parameter("boff", [1, 1], mybir.dt.int32, isOutput=False)
    out = nc.declare_dram_parameter("out", [128, t_steps, 256], F16, isOutput=True)

    # internal DRAM for collectives (v gathered in two chunks for overlap)
    vin_dA = nc.dram_tensor("vin_dA", [9 * 128 * BL], F16)
    vout_dA = nc.dram_tensor("vout_dA", [NCORES, 128, 9, BL], F16, addr_space="Shared")
    vin_dB = nc.dram_tensor("vin_dB", [9 * 128 * BL], F16)
    vout_dB = nc.dram_tensor("vout_dB", [NCORES, 128, 9, BL], F16, addr_space="Shared")
    hin_d = nc.dram_tensor("hin_d", [128 * 256], F16)
    hout_d = nc.dram_tensor("hout_d", [NCORES, 128, 256], F16, addr_space="Shared")
    rg = [list(range(NCORES))]

    with ExitStack() as ctx:
        tc = ctx.enter_context(tile.TileContext(nc))
        sing = ctx.enter_context(tc.tile_pool(name="sing", bufs=1))
        work = ctx.enter_context(tc.tile_pool(name="work", bufs=2))
        psL = ctx.enter_context(tc.tile_pool(name="psL", bufs=1, space="PSUM"))
        psD = ctx.enter_context(tc.tile_pool(name="psD", bufs=1, space="PSUM"))
        psV = ctx.enter_context(tc.tile_pool(name="psV", bufs=2, space="PSUM"))
        psG = ctx.enter_context(tc.tile_pool(name="psG", bufs=1, space="PSUM"))

        x_sb = sing.tile([128, BL * 2 * 6 * 128], F16)
        wat_sb = sing.tile([128, 8 * 768], F16)
        wgs_sb = sing.tile([128, 26 * 4 * 128], F16)
        boff_sb = sing.tile([1, 1], mybir.dt.int32)
        nc.sync.dma_start(out=boff_sb, in_=boff[:])
        nc.sync.dma_start(out=wat_sb, in_=wat[:])
        nc.sync.dma_start(out=wgs_sb, in_=wgs[:])
        nc.scalar.dma_start(out=x_sb[:, : BL * 6 * 128], in_=xq[:][:, : BL * 6 * 128])
        nc.gpsimd.dma_start(out=x_sb[:, BL * 6 * 128 :], in_=xq[:][:, BL * 6 * 128 :])

        ones_l = sing.tile([128, 1], F16)
        nc.vector.memset(ones_l, 1.0)
        ones_c = sing.tile([1, 128], F32)
        nc.vector.memset(ones_c, 1.0)

        hT_full = sing.tile([128, 8, 256], F16)
        nc.gpsimd.memset(hT_full, 0)
        hT_own = sing.tile([128, 8, BL], F16)
        nc.gpsimd.memset(hT_own, 0)
        vT_fullA = sing.tile([128, 8, 9, BL], F16)
        vT_fullB = sing.tile([128, 8, 9, BL], F16)
        c_st = sing.tile([128, 256], F32)
        nc.gpsimd.memset(c_st, 0.0)

        # own-batch offset register on gpsimd (drives the dynamic h-slice DMA)
        boff_reg = ctx.enter_context(nc.gpsimd.register("boff_reg"))
        nc.gpsimd.load(boff_reg, boff_sb[0:1, 0:1])
        boff_val = nc.gpsimd.snap(boff_reg)

        for t in range(t_steps):
            # [P1] logits^T for OWN batches: psum_l[:, mt*32 + b']
            psum_l = psL.tile([128, 6 * BL], F32, tag="psl")
            for mt in range(6):
                for kt in range(8):
                    nc.tensor.matmul(
                        psum_l[:, mt * BL : (mt + 1) * BL],
                        wat_sb[:, kt * 768 + mt * 128 : kt * 768 + (mt + 1) * 128],
                        hT_own[:, kt, :],
                        start=(kt == 0),
                        stop=(kt == 7),
                    )

            # [A1] e = exp(logits) unnormalized -> fp16 (|logit| < ~1 so safe)
            e16 = work.tile([128, 6 * BL], F16, tag="e16")
            nc.scalar.activation(e16, psum_l, AF.Exp)

            # [P3] denominators for own batches: psum_den[0, i*32+b']
            psum_den = psD.tile([1, 96], F32, tag="psden")
            ev = e16.rearrange("p (i l b) -> p l i b", i=3, l=2)
            for lt in range(2):
                nc.tensor.matmul(
                    psum_den.rearrange("o (i b) -> o i b", i=3),
                    ones_l,
                    ev[:, lt],
                    start=(lt == 0),
                    stop=(lt == 1),
                    skip_group_check=True,
                )

            # [P5] matvec: v^T own batches, x stationary, e16 moving.
            # Chunk A = dt 0..2 (v-dims q 0..8), chunk B = dt 3..5.
            # AllGather of chunk A overlaps matvec of chunk B + gates h-part;
            # AllGather of chunk B overlaps unpack-A + gates vA-part.
            a_v = e16.rearrange("p (i l b) -> p l b i", i=3, l=2)
            vT_own = work.tile([128, 6 * 3 * BL], F16, tag="vTown")
            recip = work.tile([1, 96], F32, tag="rec")
            psum_bc = psD.tile([128, 96], F32, tag="psbc")
            sb_bc = work.tile([128, 96], F32, tag="sbbc")

            def matvec_chunk(dts):
                for dt_i in dts:
                    pv = psV.tile([128, 3 * BL], F32, tag="pv")
                    pvv = pv.rearrange("p (i b) -> p b i", i=3)
                    for b in range(BL):
                        for kt in range(2):
                            nc.tensor.matmul(
                                pvv[:, b, :],
                                x_sb[
                                    :,
                                    ((b * 2 + kt) * 6 + dt_i) * 128 : ((b * 2 + kt) * 6 + dt_i + 1) * 128,
                                ],
                                a_v[:, kt, b, :],
                                start=(kt == 0),
                                stop=(kt == 1),
                                skip_group_check=True,
                            )
                    if dt_i == 0:
                        # normalization factors: recip on DVE (concurrent
                        # with matvec), broadcast down partitions via matmul
                        nc.vector.reciprocal(recip, psum_den)
                        nc.tensor.matmul(
                            psum_bc, ones_c, recip, start=True, stop=True,
                            skip_group_check=True,
                        )
                        nc.scalar.copy(sb_bc, psum_bc)
                    # drain: v = (e @ x) * recip -> fp16
                    nc.vector.tensor_mul(
                        vT_own[:, dt_i * 96 : (dt_i + 1) * 96], pv, sb_bc
                    )

            matvec_chunk([0, 1, 2])
            # [D1/C1] AllGather v^T chunk A
            nc.scalar.dma_start(
                out=vin_dA[:].rearrange("(p c) -> p c", p=128),
                in_=vT_own[:, 0 : 3 * 96],
            )
            nc.gpsimd.collective_compute(
                "AllGather", mybir.AluOpType.bypass, replica_groups=rg,
                ins=[vin_dA[:]], outs=[vout_dA[:]],
            )

            matvec_chunk([3, 4, 5])
            # AllGather v^T chunk B
            nc.scalar.dma_start(
                out=vin_dB[:].rearrange("(p c) -> p c", p=128),
                in_=vT_own[:, 3 * 96 : 6 * 96],
            )
            nc.gpsimd.collective_compute(
                "AllGather", mybir.AluOpType.bypass, replica_groups=rg,
                ins=[vin_dB[:]], outs=[vout_dB[:]],
            )

            # [P2] gates: grouped by psum bank (gA = i|f, gB = g|o) so that
            # sig_if can start before the gB half finishes.
            psum_gA = psG.tile([128, 512], F32, tag="gA")
            psum_gB = psG.tile([128, 512], F32, tag="gB")

            def gates_mms(kt_range, rhs_of_kt, gpair, is_first, is_last):
                ps = psum_gA if gpair == 0 else psum_gB
                for kt in kt_range:
                    rhs = rhs_of_kt(kt)
                    for g in (2 * gpair, 2 * gpair + 1):
                        nc.tensor.matmul(
                            ps[:, (g % 2) * 256 : (g % 2) * 256 + 256],
                            wgs_sb[:, (kt * 4 + g) * 128 : (kt * 4 + g + 1) * 128],
                            rhs,
                            # start=True clears has_written for the WHOLE
                            # bank: only the first gate per bank may set it
                            start=(kt == is_first and g % 2 == 0),
                            stop=(kt == is_last),
                            skip_group_check=True,
                        )

            h_rhs = lambda kt: hT_full[:, kt - 18, :]
            vA_rhs = lambda kt: vT_fullA[:, :, kt, :]
            vB_rhs = lambda kt: vT_fullB[:, :, kt - 9, :]

            # h-part (runs during AG-A alongside matvec chunk B)
            gates_mms(range(18, 26), h_rhs, 0, 18, None)
            gates_mms(range(18, 26), h_rhs, 1, 18, None)

            # [D2] unpack gathered v chunk A (contiguous 576B runs per (r,p))
            nc.sync.dma_start(
                out=vT_fullA, in_=vout_dA[:].transpose([1, 0, 2, 3])
            )
            # vA-part (runs during AG-B)
            gates_mms(range(0, 9), vA_rhs, 0, None, None)
            gates_mms(range(0, 9), vA_rhs, 1, None, None)

            # unpack chunk B + vB-part; gA bank finishes first so sig_if
            # overlaps the gB half
            nc.sync.dma_start(
                out=vT_fullB, in_=vout_dB[:].transpose([1, 0, 2, 3])
            )
            gates_mms(range(9, 18), vB_rhs, 0, None, 17)
            sig_if = work.tile([128, 512], F32, tag="sif")
            nc.scalar.activation(sig_if, psum_gA, AF.Sigmoid)
            t2 = work.tile([128, 256], F32, tag="t2")
            nc.vector.tensor_mul(t2, sig_if[:, 256:512], c_st)
            gates_mms(range(9, 18), vB_rhs, 1, None, 17)

            # [A3/V3] LSTM elementwise (partitions = own H dims, free = B)
            tg = work.tile([128, 256], F32, tag="tg")
            nc.scalar.activation(tg, psum_gB[:, 0:256], AF.Tanh)
            sig_o = work.tile([128, 256], F32, tag="so")
            nc.scalar.activation(sig_o, psum_gB[:, 256:512], AF.Sigmoid)

            t1 = work.tile([128, 256], F32, tag="t1")
            nc.vector.tensor_mul(t1, sig_if[:, 0:256], tg)
            nc.vector.tensor_add(c_st, t1, t2)
            tc_t = work.tile([128, 256], F32, tag="tct")
            nc.scalar.activation(tc_t, c_st, AF.Tanh)
            h16 = work.tile([128, 256], F16, tag="h16")
            nc.vector.tensor_mul(h16, sig_o, tc_t)

            # [D3] write own H-slice of h_t: out[p, t, B]
            nc.gpsimd.dma_start(out=out[:, t, :], in_=h16)

            # [D4/C2/D5] AllGather h^T
            if t < t_steps - 1:
                nc.scalar.dma_start(
                    out=hin_d[:].rearrange("(p c) -> p c", p=128), in_=h16
                )
                nc.gpsimd.collective_compute(
                    "AllGather",
                    mybir.AluOpType.bypass,
                    replica_groups=rg,
                    ins=[hin_d[:]],
                    outs=[hout_d[:]],
                )
                nc.gpsimd.dma_start(
                    out=hT_own,
                    in_=hout_d[:, :, bass.ds(boff_val, BL)].transpose([1, 0, 2]),
                )
                for r in range(NCORES):
                    eng = [nc.sync, nc.scalar][r % 2]
                    eng.dma_start(out=hT_full[:, r], in_=hout_d[r])

    nc.compile()
    return nc


def prep_shared(W1, W2, W3):
    Watt = np.concatenate([W1, W2, W3], axis=0)  # [768, H]
    WT = Watt.T  # [H, 768]
    wat = WT.reshape(8, 128, 768).transpose(1, 0, 2).reshape(128, 8 * 768)
    return np.ascontiguousarray(wat).astype(np.float16)


def prep_wgs(W_ih, W_hh, k):
    # lhsT tile (kt, g): [p, r] = W_cat^T[kt*128+p, g*1024 + 128k + r]
    WihT = W_ih.T  # [2304, 4096]
    rows = []
    for dt_i in range(6):
        for i in range(3):
            rows.append(WihT[i * 768 + dt_i * 128 : i * 768 + (dt_i + 1) * 128])
    rows.append(W_hh.T)  # [1024, 4096]
    WcatT = np.concatenate(rows, axis=0)  # [3328, 4096]
    cols = np.concatenate(
        [np.arange(g * 1024 + 128 * k, g * 1024 + 128 * k + 128) for g in range(4)]
    )
    Wsh = WcatT[:, cols]  # [3328, 512]
    wgs = Wsh.reshape(26, 128, 4, 128).transpose(1, 0, 2, 3).reshape(128, 26 * 4 * 128)
    return np.ascontiguousarray(wgs).astype(np.float16)


def prep_x_shard(x_shard):
    xs = x_shard.reshape(BL, 2, 128, 6, 128)
    xp = xs.transpose(2, 0, 1, 3, 4).reshape(128, BL * 2 * 6 * 128)
    return np.ascontiguousarray(xp).astype(np.float16)


_CACHE = {}


def make_in_maps(inputs):
    x = np.asarray(inputs["x"], np.float32)
    W1 = np.asarray(inputs["W1"], np.float32)
    W2 = np.asarray(inputs["W2"], np.float32)
    W3 = np.asarray(inputs["W3"], np.float32)
    W_ih = np.asarray(inputs["W_ih"], np.float32)
    W_hh = np.asarray(inputs["W_hh"], np.float32)
    wat = prep_shared(W1, W2, W3)
    in_maps = []
    for core in range(NCORES):
        in_maps.append(
            {
                "xq": prep_x_shard(x[core * BL : (core + 1) * BL]),
                "wat": wat,
                "wgs": prep_wgs(W_ih, W_hh, core),
                "boff": np.array([[core * BL]], np.int32),
            }
        )
    return in_maps


def kernel(**inputs):
    t_steps = int(np.asarray(inputs["output_size"]))
    if t_steps not in _CACHE:
        _CACHE[t_steps] = build_program(t_steps)
    nc = _CACHE[t_steps]
    in_maps = make_in_maps(inputs)

    from concourse.bass_utils import run_bass_kernel_spmd

    res = run_bass_kernel_spmd(nc, in_maps, list(range(NCORES)))
    # core k's out: [128, T, 256] = h[B, t, 128k+p] transposed
    outs = [
        np.asarray(res.results[i]["out"], np.float32).transpose(2, 1, 0)
        for i in range(NCORES)
    ]
    return np.concatenate(outs, axis=2)


if __name__ == "__main__":
    nc = build_program(int(sys.argv[1]) if len(sys.argv) > 1 else 1)
    print("built ok")
